# revision 8
# baseline (speedup 1.0000x reference)
"""DeepseekV2 MoE layer (M=1024, H=1024, N=1024, E=16, top-6 of 8 groups x2)
on 8 Trainium2 NeuronCores.

Sharding: expert parallelism with group-aligned placement. E=16 experts in 8
groups of 2; grouped_topk keeps the top-3 groups and top_k=6 = 3*2 takes ALL
experts of those groups. Core c owns group c (experts 2c, 2c+1): the host
routes (tiny softmax over 16 logits), shards the token set per core (the
"dispatch"), and each core runs both expert MLPs on its tokens. The
shared-expert MLP is tensor-parallel over its intermediate dim (256 of 2048
per core). The host sums the per-core partials (the "combine" step).

All GEMMs run in fp16 (fp32 PSUM accumulation; end-to-end rel err ~6e-4).
Weights are re-laid-out host-side so every contraction dim lands on SBUF
partitions.

Schedule (PE-roofline-driven, from trace analysis of the v1 kernel):
- few LARGE DMAs (one DMA's packets fan out across all 16 HW DGE queues, and
  each DMA_DIRECT2D issue costs ~600ns of Sync-queue time regardless of size)
- a dozen dummy matmuls on a zeroed scratch tile warm the PE HAM clock-gate
  (1.2->2.4 GHz) while the first real loads are still in flight
- GEMM1 streams tokens against stationary w1 chunks (N=W cols/matmul)
- GEMM2 is restructured vs v1: w2 [n,o]-chunks stationary, gtw streamed, so
  the out partition dim is a full 128 (o) instead of W%128 token chunks
- phase order GEMM1 -> shared-s1 -> shared-s2 -> GEMM2: the slow serialized
  PSUM drains of s2 ([128,1024] copies) overlap GEMM2's 22us of PE work, and
  the kernel tail after the last matmul is only one [128,W] copy + store
- PSUM->SBUF drains on the (otherwise idle) Vector engine; store DMAs issued
  from the Scalar HWDGE queue so the Sync queue only carries loads
"""
import sys

sys.path.insert(0, "/opt/trn_rl_repo")

import numpy as np

import concourse.mybir as mybir
import concourse.tile as tile
from concourse import bacc
from concourse.bass_utils import run_bass_kernel_spmd

P = 128
M = 1024          # tokens
H = 1024          # hidden
NI = 1024         # moe_intermediate
E = 16
N_GROUP = 8
TOPK_GROUP = 3
I_SH = 2048       # shared-expert intermediate (n_shared * moe_intermediate)
ISH_C = I_SH // 8  # per-core shared slice = 256

F32 = mybir.dt.float32
F16 = mybir.dt.float16
AF = mybir.ActivationFunctionType
MULT = mybir.AluOpType.mult

_PROGRAM_CACHE = {}


def _build_program(W):
    """SPMD program for one core; W = token capacity (<=512)."""
    assert 0 < W <= 512

    nc = bacc.Bacc("TRN2", target_bir_lowering=False, debug=False, num_devices=8)

    # --- per-core DRAM I/O ---
    # w1c[e*8+j] = [128 h-in-chunk, 0:1024 gate f-chunk j | 1024:2048 up f-chunk j]
    w1c = nc.dram_tensor("w1c", [16, P, 2048], F16, kind="ExternalInput").ap()
    # w2c[e] = [128 n-in-chunk, n-chunk-major o cols]: block n at cols n*1024+o
    w2c = nc.dram_tensor("w2c", [2, P, 8 * 1024], F16, kind="ExternalInput").ap()
    xt = nc.dram_tensor("xt", [P, 8 * W], F16, kind="ExternalInput").ap()
    ht = nc.dram_tensor("ht", [P, 8 * M], F16, kind="ExternalInput").ap()
    gus = nc.dram_tensor("gus", [P, 8 * 2 * ISH_C], F16, kind="ExternalInput").ap()
    dst = nc.dram_tensor("dst", [P, 2 * H], F16, kind="ExternalInput").ap()
    wab = nc.dram_tensor("wab", [P, 2 * W], F32, kind="ExternalInput").ap()
    # y[oc] = [128 o rows of chunk oc, W token cols]  (routed out, transposed)
    y = nc.dram_tensor("y", [8, P, W], F16, kind="ExternalOutput").ap()
    shared = nc.dram_tensor("shared", [M, H], F16, kind="ExternalOutput").ap()

    with tile.TileContext(nc) as tc:
        with (
            tc.tile_pool(name="persist", bufs=1) as persist,
            tc.tile_pool(name="stream", bufs=8) as stream,
            tc.tile_pool(name="work", bufs=2) as work,
            tc.tile_pool(name="drain", bufs=4) as drain,
            tc.tile_pool(name="psum", bufs=4, space="PSUM") as psum,
        ):
            # --- PE warm-up: HAM un-throttles after ~3.4us of sustained MMs;
            # run dummies on a zeroed tile while the first loads are in flight.
            zt = persist.tile([P, 640], F16, tag="zt")
            nc.gpsimd.memset(zt[:], 0.0)
            with nc.named_scope("warm"):
                wps = psum.tile([P, 1024], F32, space="PSUM", tag="acc")
                for _ in range(9):
                    nc.tensor.matmul(
                        wps[:, :512], zt[:, :P], zt[:, P:640], start=True, stop=True
                    )

            # --- loads: few large DMAs, first-needed first ---
            t_xta = persist.tile([P, 4 * W], F16, tag="xta")
            t_xtb = persist.tile([P, 4 * W], F16, tag="xtb")
            t_wab = persist.tile([P, 2 * W], F32, tag="wab")
            t_w1 = [
                stream.tile([P, 2048], F16, tag="w1", name=f"w1_{i}")
                for i in range(16)
            ]
            # xt/wab issue from the Scalar HWDGE queue in parallel with
            # the w1 stream on Sync: first-chunk data lands ~1us earlier
            nc.scalar.dma_start(out=t_xta[:], in_=xt[:, :4 * W])
            nc.sync.dma_start(out=t_w1[0][:], in_=w1c[0])
            nc.scalar.dma_start(out=t_xtb[:], in_=xt[:, 4 * W:])
            nc.sync.dma_start(out=t_w1[1][:], in_=w1c[1])
            nc.scalar.dma_start(out=t_wab[:], in_=wab)
            for i in range(2, 16):
                nc.sync.dma_start(out=t_w1[i][:], in_=w1c[i])
            t_ht = persist.tile([P, 8 * M], F16, tag="ht")
            nc.sync.dma_start(out=t_ht[:], in_=ht)
            t_gus = persist.tile([P, 8 * 2 * ISH_C], F16, tag="gus")
            nc.sync.dma_start(out=t_gus[:], in_=gus)
            t_dst = persist.tile([P, 2 * H], F16, tag="dst")
            nc.sync.dma_start(out=t_dst[:], in_=dst)
            t_w2 = persist.tile([P, 2 * 8 * 1024], F16, tag="w2")
            for e in range(2):
                nc.sync.dma_start(
                    out=t_w2[:, e * 8192:(e + 1) * 8192], in_=w2c[e]
                )

            # --- GEMM1 + silu*up*weight -> gtw [f-in-chunk, (e,j) blocks of W] ---
            t_gtw = persist.tile([P, 2 * 8 * W], F16, tag="gtw")
            with nc.named_scope("gemm1"):
                for e in range(2):
                    for j in range(8):
                        w1t = t_w1[e * 8 + j]
                        pg = psum.tile([P, W], F32, space="PSUM", tag="acc")
                        pu = psum.tile([P, W], F32, space="PSUM", tag="acc")
                        for k in range(8):
                            xk = t_xta if k < 4 else t_xtb
                            nc.tensor.matmul(
                                pg[:],
                                w1t[:, k * P:(k + 1) * P],
                                xk[:, (k % 4) * W:(k % 4 + 1) * W],
                                start=(k == 0),
                                stop=(k == 7),
                            )
                        for k in range(8):
                            xk = t_xta if k < 4 else t_xtb
                            nc.tensor.matmul(
                                pu[:],
                                w1t[:, 1024 + k * P:1024 + (k + 1) * P],
                                xk[:, (k % 4) * W:(k % 4 + 1) * W],
                                start=(k == 0),
                                stop=(k == 7),
                            )
                        sg = work.tile([P, W], F32, tag="sg")
                        nc.scalar.activation(out=sg[:], in_=pg[:], func=AF.Silu)
                        gt = work.tile([P, W], F32, tag="gt")
                        nc.vector.tensor_tensor(
                            out=gt[:], in0=sg[:], in1=pu[:], op=MULT
                        )
                        nc.vector.tensor_tensor(
                            out=t_gtw[:, (e * 8 + j) * W:(e * 8 + j + 1) * W],
                            in0=gt[:],
                            in1=t_wab[:, e * W:(e + 1) * W],
                            op=MULT,
                        )

            # --- shared expert s1 (TP slice over intermediate dim) ---
            t_gts = persist.tile([P, 2 * M], F16, tag="gts")
            with nc.named_scope("shared1"):
                for ip in range(2):  # 128-row i-slices of the 256-wide TP slice
                    ag = psum.tile([P, 1024], F32, space="PSUM", tag="acc")
                    au = psum.tile([P, 1024], F32, space="PSUM", tag="acc")
                    for mh in range(2):
                        for k in range(8):
                            nc.tensor.matmul(
                                ag[:, mh * 512:(mh + 1) * 512],
                                t_gus[:, k * 512 + ip * P:k * 512 + (ip + 1) * P],
                                t_ht[:, k * M + mh * 512:k * M + (mh + 1) * 512],
                                start=(k == 0),
                                stop=(k == 7),
                            )
                    for mh in range(2):
                        for k in range(8):
                            nc.tensor.matmul(
                                au[:, mh * 512:(mh + 1) * 512],
                                t_gus[:, k * 512 + 256 + ip * P:
                                      k * 512 + 256 + (ip + 1) * P],
                                t_ht[:, k * M + mh * 512:k * M + (mh + 1) * 512],
                                start=(k == 0),
                                stop=(k == 7),
                            )
                    ss = work.tile([P, M], F32, tag="ss")
                    nc.scalar.activation(out=ss[:], in_=ag[:], func=AF.Silu)
                    nc.vector.tensor_tensor(
                        out=t_gts[:, ip * M:(ip + 1) * M],
                        in0=ss[:],
                        in1=au[:],
                        op=MULT,
                    )

            # --- shared s2 (shared[m,o] = sum_i gts[i,m]*dst[i,o]) interleaved
            # with GEMM2 (y[o,c] = sum_{e,n} w2[e][o,n]*gtw[e,n,c]).
            # The PE queue is in-order, so GEMM2 o-chunk groups (whose inputs
            # are long since ready) are emitted between s2 m-chunk groups to
            # cover s2's drain latency; s2 drains alternate Scalar/Vector and
            # all stores issue from the (idle) Sync HWDGE queue.
            def s2_group(mc):
                acc = psum.tile(
                    [P, 1024], F32, space="PSUM", tag="acc", name=f"s2_{mc}"
                )
                for oh in range(2):
                    for ip in range(2):
                        nc.tensor.matmul(
                            acc[:, oh * 512:(oh + 1) * 512],
                            t_gts[:, ip * M + mc * P:ip * M + (mc + 1) * P],
                            t_dst[:, ip * H + oh * 512:ip * H + (oh + 1) * 512],
                            start=(ip == 0),
                            stop=(ip == 1),
                        )
                sh = drain.tile([P, H], F16, tag="sh", name=f"sh_{mc}")
                if mc % 2 == 0:
                    nc.vector.tensor_copy(out=sh[:], in_=acc[:])
                else:
                    nc.scalar.copy(out=sh[:], in_=acc[:])
                nc.sync.dma_start(out=shared[mc * P:(mc + 1) * P, :], in_=sh[:])

            def gemm2_group(oc):
                occ = psum.tile(
                    [P, W], F32, space="PSUM", tag="acc", name=f"occ_{oc}"
                )
                for e in range(2):
                    for n in range(8):
                        nc.tensor.matmul(
                            occ[:],
                            t_w2[:, e * 8192 + n * 1024 + oc * P:
                                 e * 8192 + n * 1024 + (oc + 1) * P],
                            t_gtw[:, (e * 8 + n) * W:(e * 8 + n + 1) * W],
                            start=(e == 0 and n == 0),
                            stop=(e == 1 and n == 7),
                        )
                yt = drain.tile([P, W], F16, tag="yt", name=f"yt_{oc}")
                nc.vector.tensor_copy(out=yt[:], in_=occ[:])
                nc.sync.dma_start(out=y[oc], in_=yt[:])

            with nc.named_scope("s2_gemm2"):
                gemm2_group(0)
                for pair in range(4):
                    s2_group(2 * pair)
                    s2_group(2 * pair + 1)
                    gemm2_group(pair + 1)
                for oc in range(5, 8):
                    gemm2_group(oc)

    nc.compile()
    return nc


def _get_program(W):
    if W not in _PROGRAM_CACHE:
        _PROGRAM_CACHE[W] = _build_program(W)
    return _PROGRAM_CACHE[W]


def _route(hidden_states, gate_w):
    """Numpy replica of grouped_topk: softmax -> per-group max -> top-3 groups.
    With E=16, n_group=8, topk_group=3, top_k=6, the top-6 experts are exactly
    all experts of the top-3 groups and keep their softmax scores."""
    lg = hidden_states @ gate_w.T
    lg = lg - lg.max(axis=1, keepdims=True)
    sc = np.exp(lg)
    sc /= sc.sum(axis=1, keepdims=True)
    gsc = sc.reshape(M, N_GROUP, E // N_GROUP).max(axis=2)
    top = np.argsort(-gsc, axis=1, kind="stable")[:, :TOPK_GROUP]
    gmask = np.zeros((M, N_GROUP), bool)
    np.put_along_axis(gmask, top, True, axis=1)
    return sc.astype(np.float32), gmask


def _chunk_major(a, nchunk):
    """[nchunk*P, C] -> [P, nchunk*C] with chunk k at cols k*C."""
    c = a.shape[1]
    return np.ascontiguousarray(
        a.reshape(nchunk, P, c).transpose(1, 0, 2).reshape(P, nchunk * c)
    )


def _prep_core(c, hidden, ht_l, w1, w2, sgu_t, sd_t, sc, gmask, W):
    tok = np.nonzero(gmask[:, c])[0].astype(np.int32)
    n = len(tok)
    wa = np.zeros(W, np.float32)
    wb = np.zeros(W, np.float32)
    wa[:n] = sc[tok, 2 * c]
    wb[:n] = sc[tok, 2 * c + 1]

    xp = np.zeros((W, H), np.float32)
    xp[:n] = hidden[tok]
    xtc = _chunk_major(xp.T.astype(np.float16), 8)  # [P, 8W]

    w1c = np.empty((16, P, 2048), np.float16)
    w2c = np.empty((2, P, 8 * 1024), np.float16)
    for i, e in enumerate((2 * c, 2 * c + 1)):
        # block (f_chunk j, h_chunk k): [h_in (part), f_in] = w1[e][j*128+q, k*128+p]
        w1r = (
            w1[e].reshape(16, P, 8, P).transpose(0, 3, 2, 1)
            .reshape(16, P, 8 * P).astype(np.float16)
        )
        w1c[i * 8:(i + 1) * 8, :, :1024] = w1r[:8]
        w1c[i * 8:(i + 1) * 8, :, 1024:] = w1r[8:]
        w2c[i] = _chunk_major(
            np.ascontiguousarray(w2[e].T).astype(np.float16), 8
        )

    gusc = _chunk_major(
        np.concatenate(
            (
                sgu_t[:, c * ISH_C:(c + 1) * ISH_C],
                sgu_t[:, I_SH + c * ISH_C:I_SH + (c + 1) * ISH_C],
            ),
            axis=1,
        ).astype(np.float16),
        8,
    )  # [P, 8*512]
    dstc = _chunk_major(
        sd_t[c * ISH_C:(c + 1) * ISH_C, :].astype(np.float16), 2
    )  # [P, 2H]

    return tok, {
        "w1c": w1c,
        "w2c": w2c,
        "xt": xtc,
        "ht": ht_l,
        "gus": gusc,
        "dst": dstc,
        "wab": np.ascontiguousarray(
            np.concatenate(
                (np.broadcast_to(wa, (P, W)), np.broadcast_to(wb, (P, W))), axis=1
            )
        ),
    }


def _run(inputs, trace=False):
    hidden = np.ascontiguousarray(np.asarray(inputs["hidden_states"], np.float32))
    gate_w = np.asarray(inputs["gate_w"], np.float32)
    w1 = np.asarray(inputs["w1"], np.float32)
    w2 = np.asarray(inputs["w2"], np.float32)
    sgu = np.asarray(inputs["shared_gate_up"], np.float32)
    sd = np.asarray(inputs["shared_down"], np.float32)

    sc, gmask = _route(hidden, gate_w)
    counts = gmask.sum(axis=0)
    W = int(min(512, -(-int(counts.max()) // 16) * 16))
    assert counts.max() <= W, f"capacity overflow: {counts}"

    ht_l = _chunk_major(hidden.T.astype(np.float16), 8)  # [P, 8M]
    sgu_t = np.ascontiguousarray(sgu.T)  # [H, 2*I_SH]
    sd_t = np.ascontiguousarray(sd.T)    # [I_SH, H]

    nc = _get_program(W)
    toks = []
    in_maps = []
    for c in range(8):
        tok, im = _prep_core(c, hidden, ht_l, w1, w2, sgu_t, sd_t, sc, gmask, W)
        toks.append(tok)
        in_maps.append(im)
    res = run_bass_kernel_spmd(nc, in_maps, core_ids=list(range(8)), trace=trace)

    out = np.zeros((M, H), np.float32)
    for c in range(8):
        out += res.results[c]["shared"].astype(np.float32)
        tok = toks[c]
        yt = res.results[c]["y"].reshape(8 * P, W)  # [H, W] (transposed)
        out[tok] += yt[:, :len(tok)].T.astype(np.float32)
    return out, res


def kernel(**inputs):
    out, _ = _run(inputs, trace=False)
    return out


# revision 9
# speedup vs baseline: 1.0406x; 1.0406x over previous
"""DeepseekV2 MoE layer (M=1024, H=1024, N=1024, E=16, top-6 of 8 groups x2)
on 8 Trainium2 NeuronCores.

Sharding: expert parallelism with group-aligned placement. E=16 experts in 8
groups of 2; grouped_topk keeps the top-3 groups and top_k=6 = 3*2 takes ALL
experts of those groups. Core c owns group c (experts 2c, 2c+1): the host
routes (tiny softmax over 16 logits), shards the token set per core (the
"dispatch"), and each core runs both expert MLPs on its tokens. The
shared-expert MLP is tensor-parallel over its intermediate dim (256 of 2048
per core). The host sums the per-core partials (the "combine" step).

All GEMMs run in fp16 (fp32 PSUM accumulation; end-to-end rel err ~6e-4).
Weights are re-laid-out host-side so every contraction dim lands on SBUF
partitions.

Schedule (PE-roofline-driven, from trace analysis of the v1 kernel):
- few LARGE DMAs (one DMA's packets fan out across all 16 HW DGE queues, and
  each DMA_DIRECT2D issue costs ~600ns of Sync-queue time regardless of size)
- a dozen dummy matmuls on a zeroed scratch tile warm the PE HAM clock-gate
  (1.2->2.4 GHz) while the first real loads are still in flight
- GEMM1 streams tokens against stationary w1 chunks (N=W cols/matmul)
- GEMM2 is restructured vs v1: w2 [n,o]-chunks stationary, gtw streamed, so
  the out partition dim is a full 128 (o) instead of W%128 token chunks
- phase order GEMM1 -> shared-s1 -> shared-s2 -> GEMM2: the slow serialized
  PSUM drains of s2 ([128,1024] copies) overlap GEMM2's 22us of PE work, and
  the kernel tail after the last matmul is only one [128,W] copy + store
- PSUM->SBUF drains on the (otherwise idle) Vector engine; store DMAs issued
  from the Scalar HWDGE queue so the Sync queue only carries loads
"""
import sys

sys.path.insert(0, "/opt/trn_rl_repo")

import numpy as np

import concourse.mybir as mybir
import concourse.tile as tile
from concourse import bacc
from concourse.bass_utils import run_bass_kernel_spmd

P = 128
M = 1024          # tokens
H = 1024          # hidden
NI = 1024         # moe_intermediate
E = 16
N_GROUP = 8
TOPK_GROUP = 3
I_SH = 2048       # shared-expert intermediate (n_shared * moe_intermediate)
ISH_C = I_SH // 8  # per-core shared slice = 256

F32 = mybir.dt.float32
F16 = mybir.dt.float16
AF = mybir.ActivationFunctionType
MULT = mybir.AluOpType.mult

_PROGRAM_CACHE = {}


def _build_program(W):
    """SPMD program for one core; W = token capacity (<=512)."""
    assert 0 < W <= 512

    nc = bacc.Bacc("TRN2", target_bir_lowering=False, debug=False, num_devices=8)

    # --- per-core DRAM I/O ---
    # w1c[e*8+j] = [128 h-in-chunk, 0:1024 gate f-chunk j | 1024:2048 up f-chunk j]
    w1c = nc.dram_tensor("w1c", [16, P, 2048], F16, kind="ExternalInput").ap()
    # w2c[e] = [128 n-in-chunk, n-chunk-major o cols]: block n at cols n*1024+o
    w2c = nc.dram_tensor("w2c", [2, P, 8 * 1024], F16, kind="ExternalInput").ap()
    xt = nc.dram_tensor("xt", [P, 8 * W], F16, kind="ExternalInput").ap()
    ht = nc.dram_tensor("ht", [P, 8 * M], F16, kind="ExternalInput").ap()
    gus = nc.dram_tensor("gus", [P, 8 * 2 * ISH_C], F16, kind="ExternalInput").ap()
    dst = nc.dram_tensor("dst", [P, 2 * H], F16, kind="ExternalInput").ap()
    wab = nc.dram_tensor("wab", [P, 2 * W], F32, kind="ExternalInput").ap()
    # y[oc] = [128 o rows of chunk oc, W token cols]  (routed out, transposed)
    y = nc.dram_tensor("y", [8, P, W], F16, kind="ExternalOutput").ap()
    shared = nc.dram_tensor("shared", [M, H], F16, kind="ExternalOutput").ap()

    with tile.TileContext(nc) as tc:
        with (
            tc.tile_pool(name="persist", bufs=1) as persist,
            tc.tile_pool(name="stream", bufs=8) as stream,
            tc.tile_pool(name="work", bufs=2) as work,
            tc.tile_pool(name="drain", bufs=4) as drain,
            tc.tile_pool(name="psum", bufs=4, space="PSUM") as psum,
        ):
            # --- PE warm-up: HAM un-throttles after ~3.4us of sustained MMs;
            # run dummies on a zeroed tile while the first loads are in flight.
            zt = persist.tile([P, 640], F16, tag="zt")
            nc.gpsimd.memset(zt[:], 0.0)
            with nc.named_scope("warm"):
                wps = psum.tile([P, 1024], F32, space="PSUM", tag="acc")
                for _ in range(13):
                    nc.tensor.matmul(
                        wps[:, :512], zt[:, :P], zt[:, P:640], start=True, stop=True
                    )

            # --- loads: few large DMAs, first-needed first ---
            t_xta = persist.tile([P, 4 * W], F16, tag="xta")
            t_xtb = persist.tile([P, 4 * W], F16, tag="xtb")
            t_wab = persist.tile([P, 2 * W], F32, tag="wab")
            t_w1 = [
                stream.tile([P, 2048], F16, tag="w1", name=f"w1_{i}")
                for i in range(16)
            ]
            nc.sync.dma_start(out=t_xta[:], in_=xt[:, :4 * W])
            nc.sync.dma_start(out=t_w1[0][:], in_=w1c[0])
            nc.sync.dma_start(out=t_xtb[:], in_=xt[:, 4 * W:])
            nc.sync.dma_start(out=t_w1[1][:], in_=w1c[1])
            nc.sync.dma_start(out=t_w1[2][:], in_=w1c[2])
            nc.sync.dma_start(out=t_wab[:], in_=wab)
            for i in range(3, 16):
                nc.sync.dma_start(out=t_w1[i][:], in_=w1c[i])
            t_ht = persist.tile([P, 8 * M], F16, tag="ht")
            nc.sync.dma_start(out=t_ht[:], in_=ht)
            t_gus = persist.tile([P, 8 * 2 * ISH_C], F16, tag="gus")
            nc.sync.dma_start(out=t_gus[:], in_=gus)
            t_dst = persist.tile([P, 2 * H], F16, tag="dst")
            nc.sync.dma_start(out=t_dst[:], in_=dst)
            t_w2 = persist.tile([P, 2 * 8 * 1024], F16, tag="w2")
            for e in range(2):
                nc.sync.dma_start(
                    out=t_w2[:, e * 8192:(e + 1) * 8192], in_=w2c[e]
                )

            # --- GEMM1 + silu*up*weight -> gtw [f-in-chunk, (e,j) blocks of W] ---
            t_gtw = persist.tile([P, 2 * 8 * W], F16, tag="gtw")
            with nc.named_scope("gemm1"):
                for e in range(2):
                    for j in range(8):
                        w1t = t_w1[e * 8 + j]
                        pg = psum.tile([P, W], F32, space="PSUM", tag="acc")
                        pu = psum.tile([P, W], F32, space="PSUM", tag="acc")
                        for k in range(8):
                            xk = t_xta if k < 4 else t_xtb
                            nc.tensor.matmul(
                                pg[:],
                                w1t[:, k * P:(k + 1) * P],
                                xk[:, (k % 4) * W:(k % 4 + 1) * W],
                                start=(k == 0),
                                stop=(k == 7),
                            )
                        for k in range(8):
                            xk = t_xta if k < 4 else t_xtb
                            nc.tensor.matmul(
                                pu[:],
                                w1t[:, 1024 + k * P:1024 + (k + 1) * P],
                                xk[:, (k % 4) * W:(k % 4 + 1) * W],
                                start=(k == 0),
                                stop=(k == 7),
                            )
                        sg = work.tile([P, W], F32, tag="sg")
                        nc.scalar.activation(out=sg[:], in_=pg[:], func=AF.Silu)
                        gt = work.tile([P, W], F32, tag="gt")
                        nc.vector.tensor_tensor(
                            out=gt[:], in0=sg[:], in1=pu[:], op=MULT
                        )
                        nc.vector.tensor_tensor(
                            out=t_gtw[:, (e * 8 + j) * W:(e * 8 + j + 1) * W],
                            in0=gt[:],
                            in1=t_wab[:, e * W:(e + 1) * W],
                            op=MULT,
                        )

            # --- shared expert s1 (TP slice over intermediate dim) ---
            t_gts = persist.tile([P, 2 * M], F16, tag="gts")
            with nc.named_scope("shared1"):
                for ip in range(2):  # 128-row i-slices of the 256-wide TP slice
                    ag = psum.tile([P, 1024], F32, space="PSUM", tag="acc")
                    au = psum.tile([P, 1024], F32, space="PSUM", tag="acc")
                    for mh in range(2):
                        for k in range(8):
                            nc.tensor.matmul(
                                ag[:, mh * 512:(mh + 1) * 512],
                                t_gus[:, k * 512 + ip * P:k * 512 + (ip + 1) * P],
                                t_ht[:, k * M + mh * 512:k * M + (mh + 1) * 512],
                                start=(k == 0),
                                stop=(k == 7),
                            )
                    for mh in range(2):
                        for k in range(8):
                            nc.tensor.matmul(
                                au[:, mh * 512:(mh + 1) * 512],
                                t_gus[:, k * 512 + 256 + ip * P:
                                      k * 512 + 256 + (ip + 1) * P],
                                t_ht[:, k * M + mh * 512:k * M + (mh + 1) * 512],
                                start=(k == 0),
                                stop=(k == 7),
                            )
                    ss = work.tile([P, M], F32, tag="ss")
                    nc.scalar.activation(out=ss[:], in_=ag[:], func=AF.Silu)
                    nc.vector.tensor_tensor(
                        out=t_gts[:, ip * M:(ip + 1) * M],
                        in0=ss[:],
                        in1=au[:],
                        op=MULT,
                    )

            # --- shared s2 (shared[m,o] = sum_i gts[i,m]*dst[i,o]) interleaved
            # with GEMM2 (y[o,c] = sum_{e,n} w2[e][o,n]*gtw[e,n,c]).
            # The PE queue is in-order, so GEMM2 o-chunk groups (whose inputs
            # are long since ready) are emitted between s2 m-chunk groups to
            # cover s2's drain latency; s2 drains alternate Scalar/Vector and
            # all stores issue from the (idle) Sync HWDGE queue.
            def s2_group(mc):
                acc = psum.tile(
                    [P, 1024], F32, space="PSUM", tag="acc", name=f"s2_{mc}"
                )
                for oh in range(2):
                    for ip in range(2):
                        nc.tensor.matmul(
                            acc[:, oh * 512:(oh + 1) * 512],
                            t_gts[:, ip * M + mc * P:ip * M + (mc + 1) * P],
                            t_dst[:, ip * H + oh * 512:ip * H + (oh + 1) * 512],
                            start=(ip == 0),
                            stop=(ip == 1),
                        )
                sh = drain.tile([P, H], F16, tag="sh", name=f"sh_{mc}")
                if mc % 2 == 0:
                    nc.vector.tensor_copy(out=sh[:], in_=acc[:])
                else:
                    nc.scalar.copy(out=sh[:], in_=acc[:])
                nc.sync.dma_start(out=shared[mc * P:(mc + 1) * P, :], in_=sh[:])

            def gemm2_group(oc):
                occ = psum.tile(
                    [P, W], F32, space="PSUM", tag="acc", name=f"occ_{oc}"
                )
                for e in range(2):
                    for n in range(8):
                        nc.tensor.matmul(
                            occ[:],
                            t_w2[:, e * 8192 + n * 1024 + oc * P:
                                 e * 8192 + n * 1024 + (oc + 1) * P],
                            t_gtw[:, (e * 8 + n) * W:(e * 8 + n + 1) * W],
                            start=(e == 0 and n == 0),
                            stop=(e == 1 and n == 7),
                        )
                yt = drain.tile([P, W], F16, tag="yt", name=f"yt_{oc}")
                nc.vector.tensor_copy(out=yt[:], in_=occ[:])
                nc.sync.dma_start(out=y[oc], in_=yt[:])

            with nc.named_scope("s2_gemm2"):
                gemm2_group(0)
                for pair in range(4):
                    s2_group(2 * pair)
                    s2_group(2 * pair + 1)
                    gemm2_group(pair + 1)
                for oc in range(5, 8):
                    gemm2_group(oc)

    nc.compile()
    return nc


def _get_program(W):
    if W not in _PROGRAM_CACHE:
        _PROGRAM_CACHE[W] = _build_program(W)
    return _PROGRAM_CACHE[W]


def _route(hidden_states, gate_w):
    """Numpy replica of grouped_topk: softmax -> per-group max -> top-3 groups.
    With E=16, n_group=8, topk_group=3, top_k=6, the top-6 experts are exactly
    all experts of the top-3 groups and keep their softmax scores."""
    lg = hidden_states @ gate_w.T
    lg = lg - lg.max(axis=1, keepdims=True)
    sc = np.exp(lg)
    sc /= sc.sum(axis=1, keepdims=True)
    gsc = sc.reshape(M, N_GROUP, E // N_GROUP).max(axis=2)
    top = np.argsort(-gsc, axis=1, kind="stable")[:, :TOPK_GROUP]
    gmask = np.zeros((M, N_GROUP), bool)
    np.put_along_axis(gmask, top, True, axis=1)
    return sc.astype(np.float32), gmask


def _chunk_major(a, nchunk):
    """[nchunk*P, C] -> [P, nchunk*C] with chunk k at cols k*C."""
    c = a.shape[1]
    return np.ascontiguousarray(
        a.reshape(nchunk, P, c).transpose(1, 0, 2).reshape(P, nchunk * c)
    )


def _prep_core(c, hidden, ht_l, w1, w2, sgu_t, sd_t, sc, gmask, W):
    tok = np.nonzero(gmask[:, c])[0].astype(np.int32)
    n = len(tok)
    wa = np.zeros(W, np.float32)
    wb = np.zeros(W, np.float32)
    wa[:n] = sc[tok, 2 * c]
    wb[:n] = sc[tok, 2 * c + 1]

    xp = np.zeros((W, H), np.float32)
    xp[:n] = hidden[tok]
    xtc = _chunk_major(xp.T.astype(np.float16), 8)  # [P, 8W]

    w1c = np.empty((16, P, 2048), np.float16)
    w2c = np.empty((2, P, 8 * 1024), np.float16)
    for i, e in enumerate((2 * c, 2 * c + 1)):
        # block (f_chunk j, h_chunk k): [h_in (part), f_in] = w1[e][j*128+q, k*128+p]
        w1r = (
            w1[e].reshape(16, P, 8, P).transpose(0, 3, 2, 1)
            .reshape(16, P, 8 * P).astype(np.float16)
        )
        w1c[i * 8:(i + 1) * 8, :, :1024] = w1r[:8]
        w1c[i * 8:(i + 1) * 8, :, 1024:] = w1r[8:]
        w2c[i] = _chunk_major(
            np.ascontiguousarray(w2[e].T).astype(np.float16), 8
        )

    gusc = _chunk_major(
        np.concatenate(
            (
                sgu_t[:, c * ISH_C:(c + 1) * ISH_C],
                sgu_t[:, I_SH + c * ISH_C:I_SH + (c + 1) * ISH_C],
            ),
            axis=1,
        ).astype(np.float16),
        8,
    )  # [P, 8*512]
    dstc = _chunk_major(
        sd_t[c * ISH_C:(c + 1) * ISH_C, :].astype(np.float16), 2
    )  # [P, 2H]

    return tok, {
        "w1c": w1c,
        "w2c": w2c,
        "xt": xtc,
        "ht": ht_l,
        "gus": gusc,
        "dst": dstc,
        "wab": np.ascontiguousarray(
            np.concatenate(
                (np.broadcast_to(wa, (P, W)), np.broadcast_to(wb, (P, W))), axis=1
            )
        ),
    }


def _run(inputs, trace=False):
    hidden = np.ascontiguousarray(np.asarray(inputs["hidden_states"], np.float32))
    gate_w = np.asarray(inputs["gate_w"], np.float32)
    w1 = np.asarray(inputs["w1"], np.float32)
    w2 = np.asarray(inputs["w2"], np.float32)
    sgu = np.asarray(inputs["shared_gate_up"], np.float32)
    sd = np.asarray(inputs["shared_down"], np.float32)

    sc, gmask = _route(hidden, gate_w)
    counts = gmask.sum(axis=0)
    W = int(min(512, -(-int(counts.max()) // 16) * 16))
    assert counts.max() <= W, f"capacity overflow: {counts}"

    ht_l = _chunk_major(hidden.T.astype(np.float16), 8)  # [P, 8M]
    sgu_t = np.ascontiguousarray(sgu.T)  # [H, 2*I_SH]
    sd_t = np.ascontiguousarray(sd.T)    # [I_SH, H]

    nc = _get_program(W)
    toks = []
    in_maps = []
    for c in range(8):
        tok, im = _prep_core(c, hidden, ht_l, w1, w2, sgu_t, sd_t, sc, gmask, W)
        toks.append(tok)
        in_maps.append(im)
    res = run_bass_kernel_spmd(nc, in_maps, core_ids=list(range(8)), trace=trace)

    out = np.zeros((M, H), np.float32)
    for c in range(8):
        out += res.results[c]["shared"].astype(np.float32)
        tok = toks[c]
        yt = res.results[c]["y"].reshape(8 * P, W)  # [H, W] (transposed)
        out[tok] += yt[:, :len(tok)].T.astype(np.float32)
    return out, res


def kernel(**inputs):
    out, _ = _run(inputs, trace=False)
    return out
